# revision 26
# baseline (speedup 1.0000x reference)
"""Biased multi-head attention on 8 Trainium2 NeuronCores.

Sharding: batch x head-group. Core c handles batch b = c//4 and heads
4*(c%4) .. 4*(c%4)+3 (4 of 16 heads). Q/K/V projections are column-sharded
over the core's heads, scores/softmax/AV are fully local per head, and the
output projection is row-sharded (each core contributes a partial [D, L]
that the host sums per batch).

Key-side compaction (the "sparse attention" lever): keys at padded
positions get score -1e4, i.e. softmax weight exp(-1e4) -> 0, so every
byte and flop spent on them is wasted. The host drops padded key
positions up front: x / attn_bias are compacted to the nk unpadded keys
(padded up to K' = ceil(nk/384)*384 slots; slack slots are re-masked with
-1e4). With a ~50% random mask this halves the K/V projections, the
scores/softmax/AV stream, and the attn_bias DMA (the largest input).
The result only differs from the reference by the clamped floor weight
exp(-20)/Z ~ 1e-9 the reference assigns to padded keys.

Device-side dataflow (per core):
  - x arrives pre-transposed (xT [D, L] for queries, xkT [D, K'] for
    compacted keys/values); projections contract over D on partitions.
  - Q/K are produced transposed (qT [c, l], kT [c, k']); scores are
    computed transposed, S_T[k, q] = kT.T @ qT per head (contraction 64).
    Head pairs sit on partitions 0:64 / 64:128 so the two scores matmuls
    occupy disjoint PE row groups.
  - attn_bias arrives pre-transposed and key-compacted (biasT [k', q]);
    both heads of a pair load with one 3D-AP DMA. The slack-slot mask
    folds into the exp() activation's per-partition bias. Softmax skips
    max-subtraction and the +-20 clamp (scores are ~N(0,2), exp cannot
    overflow).
  - V is produced in [k', c] layout with a ones column; the AV matmul
    (lhsT = [V | 1], rhs = exp(S_T)) accumulates O_T[c, q] and the
    denominator Z[q] (row 64) in one PSUM group.
  - 1/Z is broadcast from row 64 to 64 partitions via a 2KB DRAM bounce
    (partition-stride-0 reads are only legal from DRAM), O_T rows are
    scaled on DVE, biased on ACT, staged to a DRAM OT buffer, and the
    out-projection for each 512-wide q block runs as soon as all four
    heads have produced it.

Projections and out-projection run in float32r (fp32 with 11-bit
mantissa, single-pass full-rate PE); the scores/AV stream runs in bf16,
whose fast weight load keeps LDWEIGHTS off the critical path.
"""

import os

import numpy as np

B, L, D, H = 2, 2048, 1024, 16
dh = D // H          # 64
NCORES = 8
HPC = 4              # heads per core
P = 128

_compiled = None     # (Kp, nc): compiled module and its key-slot count
LAST_RESULT = None   # BassKernelResults of the most recent run (for profiling)


def _build(Kp):
    from contextlib import ExitStack

    import concourse.bass as bass
    import concourse.tile as tile
    from concourse import bacc, mybir
    from concourse.bass import ds, ts

    f32 = mybir.dt.float32
    f32r = mybir.dt.float32r
    f16 = mybir.dt.float16
    Act = mybir.ActivationFunctionType
    KT = Kp // P          # 128-wide key chunks
    KT3 = Kp // 384       # 384-wide key tiles for the k/v projections

    nc = bacc.Bacc("TRN2", target_bir_lowering=False, debug=False,
                   num_devices=NCORES)

    xT_d = nc.dram_tensor("xT", [D, L], f32r, kind="ExternalInput").ap()
    xkT_d = nc.dram_tensor("xkT", [D, Kp], f32r, kind="ExternalInput").ap()
    wqkT_d = nc.dram_tensor("wqkT", [D, 512], f32r, kind="ExternalInput").ap()
    wvT_d = nc.dram_tensor("wvT", [D, 256], f32r, kind="ExternalInput").ap()
    bqk_d = nc.dram_tensor("bqk", [512], f32, kind="ExternalInput").ap()
    maskT_d = nc.dram_tensor("maskT", [Kp], f32, kind="ExternalInput").ap()
    biasT_d = nc.dram_tensor("biasT", [HPC, Kp, L], f16, kind="ExternalInput").ap()
    woutT_d = nc.dram_tensor("woutT", [256, D], f32r, kind="ExternalInput").ap()
    bout_d = nc.dram_tensor("bout4", [D], f32, kind="ExternalInput").ap()
    outT_d = nc.dram_tensor("outT", [D, L], f32, kind="ExternalOutput").ap()

    with tile.TileContext(nc) as tc, ExitStack() as ctx:
        consts = ctx.enter_context(tc.tile_pool(name="consts", bufs=1))
        xp = ctx.enter_context(tc.tile_pool(name="xp", bufs=2))
        biasp = ctx.enter_context(tc.tile_pool(name="biasp", bufs=10))
        expp = ctx.enter_context(tc.tile_pool(name="expp", bufs=7))
        normp = ctx.enter_context(tc.tile_pool(name="normp", bufs=5))
        outp = ctx.enter_context(tc.tile_pool(name="outp", bufs=3))
        otlp = ctx.enter_context(tc.tile_pool(name="otlp", bufs=2))
        dramp = ctx.enter_context(tc.tile_pool(name="dramp", bufs=4,
                                               space="DRAM"))
        psS = ctx.enter_context(tc.tile_pool(name="psS", bufs=2, space="PSUM"))
        psAV = ctx.enter_context(tc.tile_pool(name="psAV", bufs=4, space="PSUM"))

        wqkT_sb = consts.tile([P, 8, 512], f32r, name="wqkT_sb", tag="wqkT_sb")
        nc.gpsimd.dma_start(wqkT_sb, wqkT_d.rearrange("(o p) m -> p o m", p=P))
        wvT_sb = consts.tile([P, 8, 256], f32r, name="wvT_sb", tag="wvT_sb")
        nc.gpsimd.dma_start(wvT_sb, wvT_d.rearrange("(o p) m -> p o m", p=P))
        woutT_sb = consts.tile([P, 2, D], f32r, name="woutT_sb", tag="woutT_sb")
        nc.gpsimd.dma_start(woutT_sb, woutT_d.rearrange("(o p) m -> p o m", p=P))
        bqk_sb = consts.tile([P, 4], f32, name="bqk_sb", tag="bqk_sb")
        nc.sync.dma_start(bqk_sb, bqk_d.rearrange("(o p) -> p o", p=P))
        maskT_sb = consts.tile([P, KT], f32, name="maskT_sb", tag="maskT_sb")
        nc.sync.dma_start(maskT_sb, maskT_d.rearrange("(o p) -> p o", p=P))
        bout_sb = consts.tile([P, 8], f32, name="bout_sb", tag="bout_sb")
        nc.sync.dma_start(bout_sb, bout_d.rearrange("(o p) -> p o", p=P))

        qT_sb = consts.tile([P, 2, L], f16, name="qT_sb", tag="qT_sb")
        kT_sb = consts.tile([P, 2, Kp], f16, name="kT_sb", tag="kT_sb")
        V_sb = consts.tile([P, KT, HPC, 65], f16, name="V_sb", tag="V_sb")
        OTd = dramp.tile([256, L], f32r, name="OTd", tag="OTd", bufs=1)

        ones_c = consts.tile([P, 1], f32, name="ones_c", tag="ones_c")
        nc.vector.memset(ones_c, 1.0)
        nc.vector.tensor_copy(
            V_sb[:, :, :, 64:65],
            ones_c[:, 0:1, None, None].to_broadcast((P, KT, HPC, 1)),
        )

        # ---- Phase 1: projections -------------------------------------
        # qT[r, l] = (W_q @ x.T)[r, l] + bq, pre-scaled by 1/8
        for lt in range(4):
            xs = xp.tile([P, 8, 512], f32r, name="xs", tag="xs")
            nc.sync.dma_start(
                xs, xT_d[:, ts(lt, 512)].rearrange("(o p) l -> p o l", p=P))
            for rt in range(2):
                ps = psS.tile([P, 512], f32, name="ps_s", tag="ps_s")
                for dc in range(8):
                    nc.tensor.matmul(
                        ps,
                        lhsT=wqkT_sb[:, dc, ts(rt, P)],
                        rhs=xs[:, dc, :],
                        start=(dc == 0), stop=(dc == 7),
                    )
                nc.scalar.activation(
                    qT_sb[:, rt, ts(lt, 512)], ps, Act.Identity,
                    bias=bqk_sb[:, rt:rt + 1], scale=0.125,
                )
        # kT[r, k'] and V[k', c] over the compacted key positions.
        # qt=0's attention steps are interleaved with the k/v projection:
        # step (qt, kt) only needs kT/V chunks up to kt, so the scores/exp
        # stream starts while later key chunks are still being projected.

        avs = {}

        def emit_kvproj(kt3):
            xks = xp.tile([P, 8, 384], f32r, name="xks", tag="xks")
            nc.sync.dma_start(
                xks, xkT_d[:, ts(kt3, 384)].rearrange("(o p) l -> p o l", p=P))
            for rt in range(2):
                ps = psS.tile([P, 512], f32, name="ps_s", tag="ps_s")[:, :384]
                for dc in range(8):
                    nc.tensor.matmul(
                        ps,
                        lhsT=wqkT_sb[:, dc, ts(2 + rt, P)],
                        rhs=xks[:, dc, :],
                        start=(dc == 0), stop=(dc == 7),
                    )
                nc.scalar.activation(
                    kT_sb[:, rt, ts(kt3, 384)], ps, Act.Identity,
                    bias=bqk_sb[:, 2 + rt:3 + rt], scale=1.0,
                )
            for l4 in range(3):
                ltv = kt3 * 3 + l4
                psv = psS.tile([P, 512], f32, name="ps_s", tag="ps_s")[:, :256]
                for dc in range(8):
                    nc.tensor.matmul(
                        psv,
                        lhsT=xks[:, dc, ts(l4, P)],
                        rhs=wvT_sb[:, dc, :],
                        start=(dc == 0), stop=(dc == 7),
                    )
                nc.vector.tensor_copy(
                    V_sb[:, ltv, :, 0:64],
                    psv.rearrange("p (h c) -> p h c", c=64),
                )

        def emit_step(qt, kt, pair):
            btw = biasp.tile([P, 2, 512], f16, name="btw", tag="btw")
            nc.sync.dma_start(
                btw,
                biasT_d[2 * pair:2 * pair + 2, ts(kt, P),
                        ts(qt, 512)].rearrange("h k q -> k h q"),
            )
            swide = psS.tile([P, 2, 512], f32, name="swide", tag="ps_s")
            for hi in range(2):
                cs = slice(64 * hi, 64 * hi + 64)
                nc.tensor.matmul(
                    swide[:, hi, :],
                    lhsT=kT_sb[cs, pair, ts(kt, P)],
                    rhs=qT_sb[cs, pair, ts(qt, 512)],
                    start=True, stop=True,
                )
            ex1 = expp.tile([P, 2, 512], f16, name="ex1", tag="ex1")
            nc.scalar.activation(
                ex1, swide, Act.Exp, bias=maskT_sb[:, kt:kt + 1], scale=1.0)
            ex2 = expp.tile([P, 2, 512], f16, name="ex2", tag="ex2")
            nc.vector.tensor_mul(ex2, ex1, btw)
            for hi in range(2):
                h = 2 * pair + hi
                nc.tensor.matmul(
                    avs[qt][h],
                    lhsT=V_sb[:, kt, h, :],
                    rhs=ex2[:, hi, :],
                    start=(kt == 0), stop=(kt == KT - 1),
                )

        avcs = {}

        def emit_evac(qt):
            # evacuate the four AV psums to SBUF at the block boundary
            # (split across ACT and DVE to keep the convoy short); the
            # normalization chain is deferred into the next block
            avcs[qt] = []
            for h in range(4):
                avc = normp.tile([65, 512], f32, name="avc", tag="avc")
                if h < 2:
                    nc.scalar.copy(avc, avs[qt][h])
                else:
                    nc.vector.tensor_copy(avc, avs[qt][h])
                avcs[qt].append(avc)

        def emit_norm(qt):
            # Z broadcast via 2KB DRAM bounce + approx reciprocal
            for h in range(4):
                avc = avcs[qt][h]
                zscr = dramp.tile([1, 512], f32, name="zscr", tag="zscr")
                nc.sync.dma_start(zscr, avc[64:65, :])
                zb = normp.tile([64, 512], f32, name="zb", tag="zb")
                nc.sync.dma_start(
                    zb,
                    bass.AP(tensor=zscr.tensor, offset=zscr.offset,
                            ap=[[0, 64], [1, 512]]),
                )
                zr = normp.tile([64, 512], f32, name="zr", tag="zr")
                nc.vector.reciprocal_approx_fast(zr, zb)
                ot2 = normp.tile([64, 512], f32r, name="ot2", tag="ot2")
                nc.vector.tensor_mul(ot2, avc[0:64, :], zr)
                nc.gpsimd.dma_start(OTd[ts(h, 64), ts(qt, 512)], ot2)

        def emit_proj(qt):
            otl = otlp.tile([P, 2, 512], f32r, name="otl", tag="otl")
            nc.gpsimd.dma_start(
                otl, OTd[:, ts(qt, 512)].rearrange("(o p) l -> p o l", p=P))
            for jt in range(8):
                ps = psS.tile([P, 512], f32, name="ps_s", tag="ps_s")
                for cc in range(2):
                    nc.tensor.matmul(
                        ps,
                        lhsT=woutT_sb[:, cc, ts(jt, P)],
                        rhs=otl[:, cc, :],
                        start=(cc == 0), stop=(cc == 1),
                    )
                osb = outp.tile([P, 512], f32, name="osb", tag="osb")
                nc.vector.tensor_scalar_add(osb, ps, bout_sb[:, jt:jt + 1])
                nc.sync.dma_start(outT_d[ts(jt, P), ts(qt, 512)], osb)

        avs[0] = [psAV.tile([65, 512], f32, name=f"av0{hh}", tag="av")
                  for hh in range(4)]
        for kt3 in range(KT3):
            emit_kvproj(kt3)
            for kt in range(3 * kt3, min(3 * kt3 + 3, KT)):
                for pair in range(2):
                    emit_step(0, kt, pair)
        emit_evac(0)
        for qt in range(1, 4):
            avs[qt] = [psAV.tile([65, 512], f32, name=f"av{qt}{hh}", tag="av")
                       for hh in range(4)]
            for kt in range(KT):
                for pair in range(2):
                    emit_step(qt, kt, pair)
                if kt == 1:
                    emit_norm(qt - 1)
                if kt == 4:
                    emit_proj(qt - 1)
            emit_evac(qt)
        emit_norm(3)
        emit_proj(3)

    nc.compile()
    return nc


def _prep_core_inputs(c, Kp, x, key_padding_mask, attn_bias, W_in, b_in,
                      W_out, b_out):
    b, hg = c // HPC, c % HPC
    hs = slice(256 * hg, 256 * hg + 256)
    f32 = np.float32
    idx = np.where(~key_padding_mask[b])[0]
    nk = len(idx)
    wq, wk, wv = W_in[0:D][hs], W_in[D:2 * D][hs], W_in[2 * D:3 * D][hs]

    xk = np.zeros((Kp, D), dtype=f32)
    xk[:nk] = x[b][idx]
    # -ln(64) headroom shift: softmax is scale-invariant, and scaling all
    # exp weights by 1/64 keeps exp(s)*exp(bias) inside fp16 range.
    maskT = np.full(Kp, -10000.0, dtype=f32)
    maskT[:nk] = -np.log(64.0)
    biasT = np.zeros((HPC, Kp, L), dtype=np.float16)
    biasT[:, :nk, :] = np.exp(
        attn_bias[b, HPC * hg:HPC * hg + HPC][:, :, idx]
        .transpose(0, 2, 1)).astype(np.float16)

    return {
        "xT": np.ascontiguousarray(x[b].T, dtype=f32),
        "xkT": np.ascontiguousarray(xk.T, dtype=f32),
        "wqkT": np.ascontiguousarray(np.concatenate([wq, wk], 0).T, dtype=f32),
        "wvT": np.ascontiguousarray(wv.T, dtype=f32),
        "bqk": np.concatenate([b_in[0:D][hs] / 8.0, b_in[D:2 * D][hs]]).astype(f32),
        "maskT": maskT,
        "biasT": biasT,
        "woutT": np.ascontiguousarray(W_out[:, hs].T, dtype=f32),
        "bout4": (b_out / float(HPC) + W_out[:, hs] @ b_in[2 * D:3 * D][hs]).astype(f32),
    }


def kernel(x, key_padding_mask, attn_bias, W_in, b_in, W_out, b_out):
    global _compiled, LAST_RESULT
    from concourse.bass_utils import run_bass_kernel_spmd

    nk_max = int((~key_padding_mask).sum(axis=1).max())
    Kp = max(384, -(-nk_max // 384) * 384)

    if _compiled is None or _compiled[0] != Kp:
        _compiled = (Kp, _build(Kp))

    in_maps = [
        _prep_core_inputs(c, Kp, x, key_padding_mask, attn_bias, W_in, b_in,
                          W_out, b_out)
        for c in range(NCORES)
    ]
    res = run_bass_kernel_spmd(
        _compiled[1], in_maps, core_ids=list(range(NCORES)),
        trace_cores=(list(range(NCORES))
                     if os.environ.get("BASS_TRACE") == "1" else None),
    )
    LAST_RESULT = res

    out = np.empty((B, L, D), dtype=np.float32)
    for b in range(B):
        acc = res.results[b * HPC]["outT"].astype(np.float64)
        for g in range(1, HPC):
            acc = acc + res.results[b * HPC + g]["outT"]
        out[b] = acc.T.astype(np.float32)
    return out


# revision 27
# speedup vs baseline: 1.0372x; 1.0372x over previous
"""Biased multi-head attention on 8 Trainium2 NeuronCores.

Sharding: batch x head-group. Core c handles batch b = c//4 and heads
4*(c%4) .. 4*(c%4)+3 (4 of 16 heads). Q/K/V projections are column-sharded
over the core's heads, scores/softmax/AV are fully local per head, and the
output projection is row-sharded (each core contributes a partial [D, L]
that the host sums per batch).

Key-side compaction (the "sparse attention" lever): keys at padded
positions get score -1e4, i.e. softmax weight exp(-1e4) -> 0, so every
byte and flop spent on them is wasted. The host drops padded key
positions up front: x / attn_bias are compacted to the nk unpadded keys
(padded up to K' = ceil(nk/384)*384 slots; slack slots are re-masked with
-1e4). With a ~50% random mask this halves the K/V projections, the
scores/softmax/AV stream, and the attn_bias DMA (the largest input).
The result only differs from the reference by the clamped floor weight
exp(-20)/Z ~ 1e-9 the reference assigns to padded keys.

Device-side dataflow (per core):
  - x arrives pre-transposed (xT [D, L] for queries, xkT [D, K'] for
    compacted keys/values); projections contract over D on partitions.
  - Q/K are produced transposed (qT [c, l], kT [c, k']); scores are
    computed transposed, S_T[k, q] = kT.T @ qT per head (contraction 64).
    Head pairs sit on partitions 0:64 / 64:128 so the two scores matmuls
    occupy disjoint PE row groups.
  - attn_bias arrives pre-transposed and key-compacted (biasT [k', q]);
    both heads of a pair load with one 3D-AP DMA. The slack-slot mask
    folds into the exp() activation's per-partition bias. Softmax skips
    max-subtraction and the +-20 clamp (scores are ~N(0,2), exp cannot
    overflow).
  - V is produced in [k', c] layout with a ones column; the AV matmul
    (lhsT = [V | 1], rhs = exp(S_T)) accumulates O_T[c, q] and the
    denominator Z[q] (row 64) in one PSUM group.
  - 1/Z is broadcast from row 64 to 64 partitions via a 2KB DRAM bounce
    (partition-stride-0 reads are only legal from DRAM), O_T rows are
    scaled on DVE, biased on ACT, staged to a DRAM OT buffer, and the
    out-projection for each 512-wide q block runs as soon as all four
    heads have produced it.

Projections and out-projection run in float32r (fp32 with 11-bit
mantissa, single-pass full-rate PE); the scores/AV stream runs in bf16,
whose fast weight load keeps LDWEIGHTS off the critical path.
"""

import os

import numpy as np

B, L, D, H = 2, 2048, 1024, 16
dh = D // H          # 64
NCORES = 8
HPC = 4              # heads per core
P = 128

_compiled = None     # (Kp, nc): compiled module and its key-slot count
LAST_RESULT = None   # BassKernelResults of the most recent run (for profiling)


def _build(Kp):
    from contextlib import ExitStack

    import concourse.bass as bass
    import concourse.tile as tile
    from concourse import bacc, mybir
    from concourse.bass import ds, ts

    f32 = mybir.dt.float32
    f32r = mybir.dt.float32r
    f16 = mybir.dt.float16
    Act = mybir.ActivationFunctionType
    KT = Kp // P          # 128-wide key chunks
    KT3 = Kp // 384       # 384-wide key tiles for the k/v projections

    nc = bacc.Bacc("TRN2", target_bir_lowering=False, debug=False,
                   num_devices=NCORES)

    xT_d = nc.dram_tensor("xT", [D, L], f32r, kind="ExternalInput").ap()
    xkT_d = nc.dram_tensor("xkT", [D, Kp], f32r, kind="ExternalInput").ap()
    wqkT_d = nc.dram_tensor("wqkT", [D, 512], f32r, kind="ExternalInput").ap()
    wvT_d = nc.dram_tensor("wvT", [D, 256], f32r, kind="ExternalInput").ap()
    bqk_d = nc.dram_tensor("bqk", [512], f32, kind="ExternalInput").ap()
    maskT_d = nc.dram_tensor("maskT", [Kp], f32, kind="ExternalInput").ap()
    biasT_d = nc.dram_tensor("biasT", [HPC, Kp, L], f16, kind="ExternalInput").ap()
    woutT_d = nc.dram_tensor("woutT", [256, D], f32r, kind="ExternalInput").ap()
    bout_d = nc.dram_tensor("bout4", [D], f32, kind="ExternalInput").ap()
    outT_d = nc.dram_tensor("outT", [D, L], f32, kind="ExternalOutput").ap()

    with tile.TileContext(nc) as tc, ExitStack() as ctx:
        consts = ctx.enter_context(tc.tile_pool(name="consts", bufs=1))
        xp = ctx.enter_context(tc.tile_pool(name="xp", bufs=2))
        biasp = ctx.enter_context(tc.tile_pool(name="biasp", bufs=10))
        expp = ctx.enter_context(tc.tile_pool(name="expp", bufs=7))
        normp = ctx.enter_context(tc.tile_pool(name="normp", bufs=5))
        outp = ctx.enter_context(tc.tile_pool(name="outp", bufs=3))
        otlp = ctx.enter_context(tc.tile_pool(name="otlp", bufs=2))
        dramp = ctx.enter_context(tc.tile_pool(name="dramp", bufs=4,
                                               space="DRAM"))
        psS = ctx.enter_context(tc.tile_pool(name="psS", bufs=2, space="PSUM"))
        psAV = ctx.enter_context(tc.tile_pool(name="psAV", bufs=4, space="PSUM"))

        wqkT_sb = consts.tile([P, 8, 512], f32r, name="wqkT_sb", tag="wqkT_sb")
        nc.gpsimd.dma_start(wqkT_sb, wqkT_d.rearrange("(o p) m -> p o m", p=P))
        wvT_sb = consts.tile([P, 8, 256], f32r, name="wvT_sb", tag="wvT_sb")
        nc.gpsimd.dma_start(wvT_sb, wvT_d.rearrange("(o p) m -> p o m", p=P))
        woutT_sb = consts.tile([P, 2, D], f32r, name="woutT_sb", tag="woutT_sb")
        nc.gpsimd.dma_start(woutT_sb, woutT_d.rearrange("(o p) m -> p o m", p=P))
        bqk_sb = consts.tile([P, 4], f32, name="bqk_sb", tag="bqk_sb")
        nc.sync.dma_start(bqk_sb, bqk_d.rearrange("(o p) -> p o", p=P))
        maskT_sb = consts.tile([P, KT], f32, name="maskT_sb", tag="maskT_sb")
        nc.sync.dma_start(maskT_sb, maskT_d.rearrange("(o p) -> p o", p=P))
        bout_sb = consts.tile([P, 8], f32, name="bout_sb", tag="bout_sb")
        nc.sync.dma_start(bout_sb, bout_d.rearrange("(o p) -> p o", p=P))

        qT_sb = consts.tile([P, 2, L], f16, name="qT_sb", tag="qT_sb")
        kT_sb = consts.tile([P, 2, Kp], f16, name="kT_sb", tag="kT_sb")
        V_sb = consts.tile([P, KT, HPC, 65], f16, name="V_sb", tag="V_sb")
        OTd = dramp.tile([256, L], f32r, name="OTd", tag="OTd", bufs=1)

        ones_c = consts.tile([P, 1], f32, name="ones_c", tag="ones_c")
        nc.vector.memset(ones_c, 1.0)
        nc.vector.tensor_copy(
            V_sb[:, :, :, 64:65],
            ones_c[:, 0:1, None, None].to_broadcast((P, KT, HPC, 1)),
        )

        # ---- Phase 1: projections -------------------------------------
        # qT[r, l] = (W_q @ x.T)[r, l] + bq, pre-scaled by 1/8
        for lt in range(4):
            xs = xp.tile([P, 8, 512], f32r, name="xs", tag="xs")
            nc.sync.dma_start(
                xs, xT_d[:, ts(lt, 512)].rearrange("(o p) l -> p o l", p=P))
            for rt in range(2):
                ps = psS.tile([P, 512], f32, name="ps_s", tag="ps_s")
                for dc in range(8):
                    nc.tensor.matmul(
                        ps,
                        lhsT=wqkT_sb[:, dc, ts(rt, P)],
                        rhs=xs[:, dc, :],
                        start=(dc == 0), stop=(dc == 7),
                    )
                nc.scalar.activation(
                    qT_sb[:, rt, ts(lt, 512)], ps, Act.Identity,
                    bias=bqk_sb[:, rt:rt + 1], scale=0.125,
                )
        # kT[r, k'] and V[k', c] over the compacted key positions.
        # qt=0's attention steps are interleaved with the k/v projection:
        # step (qt, kt) only needs kT/V chunks up to kt, so the scores/exp
        # stream starts while later key chunks are still being projected.

        avs = {}

        def emit_kvproj(kt3):
            xks = xp.tile([P, 8, 384], f32r, name="xks", tag="xks")
            nc.sync.dma_start(
                xks, xkT_d[:, ts(kt3, 384)].rearrange("(o p) l -> p o l", p=P))
            for rt in range(2):
                ps = psS.tile([P, 512], f32, name="ps_s", tag="ps_s")[:, :384]
                for dc in range(8):
                    nc.tensor.matmul(
                        ps,
                        lhsT=wqkT_sb[:, dc, ts(2 + rt, P)],
                        rhs=xks[:, dc, :],
                        start=(dc == 0), stop=(dc == 7),
                    )
                nc.scalar.activation(
                    kT_sb[:, rt, ts(kt3, 384)], ps, Act.Identity,
                    bias=bqk_sb[:, 2 + rt:3 + rt], scale=1.0,
                )
            for l4 in range(3):
                ltv = kt3 * 3 + l4
                psv = psS.tile([P, 512], f32, name="ps_s", tag="ps_s")[:, :256]
                for dc in range(8):
                    nc.tensor.matmul(
                        psv,
                        lhsT=xks[:, dc, ts(l4, P)],
                        rhs=wvT_sb[:, dc, :],
                        start=(dc == 0), stop=(dc == 7),
                    )
                nc.vector.tensor_copy(
                    V_sb[:, ltv, :, 0:64],
                    psv.rearrange("p (h c) -> p h c", c=64),
                )

        def emit_step(qt, kt, pair):
            btw = biasp.tile([P, 2, 512], f16, name="btw", tag="btw")
            nc.sync.dma_start(
                btw,
                biasT_d[2 * pair:2 * pair + 2, ts(kt, P),
                        ts(qt, 512)].rearrange("h k q -> k h q"),
            )
            swide = psS.tile([P, 2, 512], f32, name="swide", tag="ps_s")
            for hi in range(2):
                cs = slice(64 * hi, 64 * hi + 64)
                nc.tensor.matmul(
                    swide[:, hi, :],
                    lhsT=kT_sb[cs, pair, ts(kt, P)],
                    rhs=qT_sb[cs, pair, ts(qt, 512)],
                    start=True, stop=True,
                )
            ex1 = expp.tile([P, 2, 512], f16, name="ex1", tag="ex1")
            nc.scalar.activation(
                ex1, swide, Act.Exp, bias=maskT_sb[:, kt:kt + 1], scale=1.0)
            ex2 = expp.tile([P, 2, 512], f16, name="ex2", tag="ex2")
            nc.vector.tensor_mul(ex2, ex1, btw)
            for hi in range(2):
                h = 2 * pair + hi
                nc.tensor.matmul(
                    avs[qt][h],
                    lhsT=V_sb[:, kt, h, :],
                    rhs=ex2[:, hi, :],
                    start=(kt == 0), stop=(kt == KT - 1),
                )

        avcs = {}

        def emit_evac(qt):
            # evacuate the four AV psums to SBUF at the block boundary
            # (split across ACT and DVE to keep the convoy short); the
            # normalization chain is deferred into the next block
            avcs[qt] = []
            for h in range(4):
                avc = normp.tile([65, 512], f32, name="avc", tag="avc")
                if h < 2:
                    nc.scalar.copy(avc, avs[qt][h])
                else:
                    nc.vector.tensor_copy(avc, avs[qt][h])
                avcs[qt].append(avc)

        def emit_norm(qt):
            # Z broadcast via 2KB DRAM bounce + approx reciprocal
            for h in range(4):
                avc = avcs[qt][h]
                zscr = dramp.tile([1, 512], f32, name="zscr", tag="zscr")
                nc.sync.dma_start(zscr, avc[64:65, :])
                zb = normp.tile([64, 512], f32, name="zb", tag="zb")
                nc.sync.dma_start(
                    zb,
                    bass.AP(tensor=zscr.tensor, offset=zscr.offset,
                            ap=[[0, 64], [1, 512]]),
                )
                zr = normp.tile([64, 512], f32, name="zr", tag="zr")
                nc.vector.reciprocal_approx_fast(zr, zb)
                ot2 = normp.tile([64, 512], f32r, name="ot2", tag="ot2")
                nc.vector.tensor_mul(ot2, avc[0:64, :], zr)
                nc.gpsimd.dma_start(OTd[ts(h, 64), ts(qt, 512)], ot2)

        def emit_proj(qt):
            otl = otlp.tile([P, 2, 512], f32r, name="otl", tag="otl")
            nc.gpsimd.dma_start(
                otl, OTd[:, ts(qt, 512)].rearrange("(o p) l -> p o l", p=P))
            for jt in range(8):
                ps = psS.tile([P, 512], f32, name="ps_s", tag="ps_s")
                for cc in range(2):
                    nc.tensor.matmul(
                        ps,
                        lhsT=woutT_sb[:, cc, ts(jt, P)],
                        rhs=otl[:, cc, :],
                        start=(cc == 0), stop=(cc == 1),
                    )
                osb = outp.tile([P, 512], f32, name="osb", tag="osb")
                nc.vector.tensor_scalar_add(osb, ps, bout_sb[:, jt:jt + 1])
                nc.sync.dma_start(outT_d[ts(jt, P), ts(qt, 512)], osb)

        avs[0] = [psAV.tile([65, 512], f32, name=f"av0{hh}", tag="av")
                  for hh in range(4)]
        emit_kvproj(0)
        for kt3 in range(KT3):
            if kt3 + 1 < KT3:
                emit_kvproj(kt3 + 1)
            for kt in range(3 * kt3, min(3 * kt3 + 3, KT)):
                for pair in range(2):
                    emit_step(0, kt, pair)
        emit_evac(0)
        for qt in range(1, 4):
            avs[qt] = [psAV.tile([65, 512], f32, name=f"av{qt}{hh}", tag="av")
                       for hh in range(4)]
            for kt in range(KT):
                for pair in range(2):
                    emit_step(qt, kt, pair)
                if kt == 0:
                    emit_norm(qt - 1)
                if kt == 5:
                    emit_proj(qt - 1)
            emit_evac(qt)
        emit_norm(3)
        emit_proj(3)

    nc.compile()
    return nc


def _prep_core_inputs(c, Kp, x, key_padding_mask, attn_bias, W_in, b_in,
                      W_out, b_out):
    b, hg = c // HPC, c % HPC
    hs = slice(256 * hg, 256 * hg + 256)
    f32 = np.float32
    idx = np.where(~key_padding_mask[b])[0]
    nk = len(idx)
    wq, wk, wv = W_in[0:D][hs], W_in[D:2 * D][hs], W_in[2 * D:3 * D][hs]

    xk = np.zeros((Kp, D), dtype=f32)
    xk[:nk] = x[b][idx]
    # -ln(64) headroom shift: softmax is scale-invariant, and scaling all
    # exp weights by 1/64 keeps exp(s)*exp(bias) inside fp16 range.
    maskT = np.full(Kp, -10000.0, dtype=f32)
    maskT[:nk] = -np.log(64.0)
    biasT = np.zeros((HPC, Kp, L), dtype=np.float16)
    biasT[:, :nk, :] = np.exp(
        attn_bias[b, HPC * hg:HPC * hg + HPC][:, :, idx]
        .transpose(0, 2, 1)).astype(np.float16)

    return {
        "xT": np.ascontiguousarray(x[b].T, dtype=f32),
        "xkT": np.ascontiguousarray(xk.T, dtype=f32),
        "wqkT": np.ascontiguousarray(np.concatenate([wq, wk], 0).T, dtype=f32),
        "wvT": np.ascontiguousarray(wv.T, dtype=f32),
        "bqk": np.concatenate([b_in[0:D][hs] / 8.0, b_in[D:2 * D][hs]]).astype(f32),
        "maskT": maskT,
        "biasT": biasT,
        "woutT": np.ascontiguousarray(W_out[:, hs].T, dtype=f32),
        "bout4": (b_out / float(HPC) + W_out[:, hs] @ b_in[2 * D:3 * D][hs]).astype(f32),
    }


def kernel(x, key_padding_mask, attn_bias, W_in, b_in, W_out, b_out):
    global _compiled, LAST_RESULT
    from concourse.bass_utils import run_bass_kernel_spmd

    nk_max = int((~key_padding_mask).sum(axis=1).max())
    Kp = max(384, -(-nk_max // 384) * 384)

    if _compiled is None or _compiled[0] != Kp:
        _compiled = (Kp, _build(Kp))

    in_maps = [
        _prep_core_inputs(c, Kp, x, key_padding_mask, attn_bias, W_in, b_in,
                          W_out, b_out)
        for c in range(NCORES)
    ]
    res = run_bass_kernel_spmd(
        _compiled[1], in_maps, core_ids=list(range(NCORES)),
        trace_cores=(list(range(NCORES))
                     if os.environ.get("BASS_TRACE") == "1" else None),
    )
    LAST_RESULT = res

    out = np.empty((B, L, D), dtype=np.float32)
    for b in range(B):
        acc = res.results[b * HPC]["outT"].astype(np.float64)
        for g in range(1, HPC):
            acc = acc + res.results[b * HPC + g]["outT"]
        out[b] = acc.T.astype(np.float32)
    return out


# revision 28
# speedup vs baseline: 1.0376x; 1.0003x over previous
"""Biased multi-head attention on 8 Trainium2 NeuronCores.

Sharding: batch x head-group. Core c handles batch b = c//4 and heads
4*(c%4) .. 4*(c%4)+3 (4 of 16 heads). Q/K/V projections are column-sharded
over the core's heads, scores/softmax/AV are fully local per head, and the
output projection is row-sharded (each core contributes a partial [D, L]
that the host sums per batch).

Key-side compaction (the "sparse attention" lever): keys at padded
positions get score -1e4, i.e. softmax weight exp(-1e4) -> 0, so every
byte and flop spent on them is wasted. The host drops padded key
positions up front: x / attn_bias are compacted to the nk unpadded keys
(padded up to K' = ceil(nk/384)*384 slots; slack slots are re-masked with
-1e4). With a ~50% random mask this halves the K/V projections, the
scores/softmax/AV stream, and the attn_bias DMA (the largest input).
The result only differs from the reference by the clamped floor weight
exp(-20)/Z ~ 1e-9 the reference assigns to padded keys.

Device-side dataflow (per core):
  - x arrives pre-transposed (xT [D, L] for queries, xkT [D, K'] for
    compacted keys/values); projections contract over D on partitions.
  - Q/K are produced transposed (qT [c, l], kT [c, k']); scores are
    computed transposed, S_T[k, q] = kT.T @ qT per head (contraction 64).
    Head pairs sit on partitions 0:64 / 64:128 so the two scores matmuls
    occupy disjoint PE row groups.
  - attn_bias arrives pre-transposed and key-compacted (biasT [k', q]);
    both heads of a pair load with one 3D-AP DMA. The slack-slot mask
    folds into the exp() activation's per-partition bias. Softmax skips
    max-subtraction and the +-20 clamp (scores are ~N(0,2), exp cannot
    overflow).
  - V is produced in [k', c] layout with a ones column; the AV matmul
    (lhsT = [V | 1], rhs = exp(S_T)) accumulates O_T[c, q] and the
    denominator Z[q] (row 64) in one PSUM group.
  - 1/Z is broadcast from row 64 to 64 partitions via a 2KB DRAM bounce
    (partition-stride-0 reads are only legal from DRAM), O_T rows are
    scaled on DVE, biased on ACT, staged to a DRAM OT buffer, and the
    out-projection for each 512-wide q block runs as soon as all four
    heads have produced it.

Projections and out-projection run in float32r (fp32 with 11-bit
mantissa, single-pass full-rate PE); the scores/AV stream runs in bf16,
whose fast weight load keeps LDWEIGHTS off the critical path.
"""

import os

import numpy as np

B, L, D, H = 2, 2048, 1024, 16
dh = D // H          # 64
NCORES = 8
HPC = 4              # heads per core
P = 128

_compiled = None     # (Kp, nc): compiled module and its key-slot count
LAST_RESULT = None   # BassKernelResults of the most recent run (for profiling)


def _build(Kp):
    from contextlib import ExitStack

    import concourse.bass as bass
    import concourse.tile as tile
    from concourse import bacc, mybir
    from concourse.bass import ds, ts

    f32 = mybir.dt.float32
    f32r = mybir.dt.float32r
    f16 = mybir.dt.float16
    Act = mybir.ActivationFunctionType
    KT = Kp // P          # 128-wide key chunks
    KT3 = Kp // 384       # 384-wide key tiles for the k/v projections

    nc = bacc.Bacc("TRN2", target_bir_lowering=False, debug=False,
                   num_devices=NCORES)

    xT_d = nc.dram_tensor("xT", [D, L], f32r, kind="ExternalInput").ap()
    xkT_d = nc.dram_tensor("xkT", [D, Kp], f32r, kind="ExternalInput").ap()
    wqkT_d = nc.dram_tensor("wqkT", [D, 512], f32r, kind="ExternalInput").ap()
    wvT_d = nc.dram_tensor("wvT", [D, 256], f32r, kind="ExternalInput").ap()
    bqk_d = nc.dram_tensor("bqk", [512], f32, kind="ExternalInput").ap()
    maskT_d = nc.dram_tensor("maskT", [Kp], f32, kind="ExternalInput").ap()
    biasT_d = nc.dram_tensor("biasT", [HPC, Kp, L], f16, kind="ExternalInput").ap()
    woutT_d = nc.dram_tensor("woutT", [256, D], f32r, kind="ExternalInput").ap()
    bout_d = nc.dram_tensor("bout4", [D], f32, kind="ExternalInput").ap()
    outT_d = nc.dram_tensor("outT", [D, L], f32, kind="ExternalOutput").ap()

    with tile.TileContext(nc) as tc, ExitStack() as ctx:
        consts = ctx.enter_context(tc.tile_pool(name="consts", bufs=1))
        xp = ctx.enter_context(tc.tile_pool(name="xp", bufs=2))
        biasp = ctx.enter_context(tc.tile_pool(name="biasp", bufs=10))
        expp = ctx.enter_context(tc.tile_pool(name="expp", bufs=7))
        normp = ctx.enter_context(tc.tile_pool(name="normp", bufs=5))
        outp = ctx.enter_context(tc.tile_pool(name="outp", bufs=3))
        otlp = ctx.enter_context(tc.tile_pool(name="otlp", bufs=2))
        dramp = ctx.enter_context(tc.tile_pool(name="dramp", bufs=4,
                                               space="DRAM"))
        psS = ctx.enter_context(tc.tile_pool(name="psS", bufs=2, space="PSUM"))
        psAV = ctx.enter_context(tc.tile_pool(name="psAV", bufs=4, space="PSUM"))

        wqkT_sb = consts.tile([P, 8, 512], f32r, name="wqkT_sb", tag="wqkT_sb")
        nc.gpsimd.dma_start(wqkT_sb, wqkT_d.rearrange("(o p) m -> p o m", p=P))
        wvT_sb = consts.tile([P, 8, 256], f32r, name="wvT_sb", tag="wvT_sb")
        nc.gpsimd.dma_start(wvT_sb, wvT_d.rearrange("(o p) m -> p o m", p=P))
        woutT_sb = consts.tile([P, 2, D], f32r, name="woutT_sb", tag="woutT_sb")
        nc.gpsimd.dma_start(woutT_sb, woutT_d.rearrange("(o p) m -> p o m", p=P))
        bqk_sb = consts.tile([P, 4], f32, name="bqk_sb", tag="bqk_sb")
        nc.sync.dma_start(bqk_sb, bqk_d.rearrange("(o p) -> p o", p=P))
        maskT_sb = consts.tile([P, KT], f32, name="maskT_sb", tag="maskT_sb")
        nc.sync.dma_start(maskT_sb, maskT_d.rearrange("(o p) -> p o", p=P))
        bout_sb = consts.tile([P, 8], f32, name="bout_sb", tag="bout_sb")
        nc.sync.dma_start(bout_sb, bout_d.rearrange("(o p) -> p o", p=P))

        qT_sb = consts.tile([P, 2, L], f16, name="qT_sb", tag="qT_sb")
        kT_sb = consts.tile([P, 2, Kp], f16, name="kT_sb", tag="kT_sb")
        V_sb = consts.tile([P, KT, HPC, 65], f16, name="V_sb", tag="V_sb")
        OTd = dramp.tile([256, L], f32r, name="OTd", tag="OTd", bufs=1)

        ones_c = consts.tile([P, 1], f32, name="ones_c", tag="ones_c")
        nc.vector.memset(ones_c, 1.0)
        nc.vector.tensor_copy(
            V_sb[:, :, :, 64:65],
            ones_c[:, 0:1, None, None].to_broadcast((P, KT, HPC, 1)),
        )

        # ---- Phase 1: projections -------------------------------------
        # qT[r, l] = (W_q @ x.T)[r, l] + bq, pre-scaled by 1/8
        for lt in range(4):
            xs = xp.tile([P, 8, 512], f32r, name="xs", tag="xs")
            nc.sync.dma_start(
                xs, xT_d[:, ts(lt, 512)].rearrange("(o p) l -> p o l", p=P))
            for rt in range(2):
                ps = psS.tile([P, 512], f32, name="ps_s", tag="ps_s")
                for dc in range(8):
                    nc.tensor.matmul(
                        ps,
                        lhsT=wqkT_sb[:, dc, ts(rt, P)],
                        rhs=xs[:, dc, :],
                        start=(dc == 0), stop=(dc == 7),
                    )
                nc.scalar.activation(
                    qT_sb[:, rt, ts(lt, 512)], ps, Act.Identity,
                    bias=bqk_sb[:, rt:rt + 1], scale=0.125,
                )
        # kT[r, k'] and V[k', c] over the compacted key positions.
        # qt=0's attention steps are interleaved with the k/v projection:
        # step (qt, kt) only needs kT/V chunks up to kt, so the scores/exp
        # stream starts while later key chunks are still being projected.

        avs = {}

        def emit_kvproj(kt3):
            xks = xp.tile([P, 8, 384], f32r, name="xks", tag="xks")
            nc.sync.dma_start(
                xks, xkT_d[:, ts(kt3, 384)].rearrange("(o p) l -> p o l", p=P))
            for rt in range(2):
                ps = psS.tile([P, 512], f32, name="ps_s", tag="ps_s")[:, :384]
                for dc in range(8):
                    nc.tensor.matmul(
                        ps,
                        lhsT=wqkT_sb[:, dc, ts(2 + rt, P)],
                        rhs=xks[:, dc, :],
                        start=(dc == 0), stop=(dc == 7),
                    )
                nc.scalar.activation(
                    kT_sb[:, rt, ts(kt3, 384)], ps, Act.Identity,
                    bias=bqk_sb[:, 2 + rt:3 + rt], scale=1.0,
                )
            for l4 in range(3):
                ltv = kt3 * 3 + l4
                psv = psS.tile([P, 512], f32, name="ps_s", tag="ps_s")[:, :256]
                for dc in range(8):
                    nc.tensor.matmul(
                        psv,
                        lhsT=xks[:, dc, ts(l4, P)],
                        rhs=wvT_sb[:, dc, :],
                        start=(dc == 0), stop=(dc == 7),
                    )
                nc.vector.tensor_copy(
                    V_sb[:, ltv, :, 0:64],
                    psv.rearrange("p (h c) -> p h c", c=64),
                )

        def emit_step(qt, kt, pair):
            btw = biasp.tile([P, 2, 512], f16, name="btw", tag="btw")
            eng = nc.sync if pair == 0 else nc.gpsimd
            eng.dma_start(
                btw,
                biasT_d[2 * pair:2 * pair + 2, ts(kt, P),
                        ts(qt, 512)].rearrange("h k q -> k h q"),
            )
            swide = psS.tile([P, 2, 512], f32, name="swide", tag="ps_s")
            for hi in range(2):
                cs = slice(64 * hi, 64 * hi + 64)
                nc.tensor.matmul(
                    swide[:, hi, :],
                    lhsT=kT_sb[cs, pair, ts(kt, P)],
                    rhs=qT_sb[cs, pair, ts(qt, 512)],
                    start=True, stop=True,
                )
            ex1 = expp.tile([P, 2, 512], f16, name="ex1", tag="ex1")
            nc.scalar.activation(
                ex1, swide, Act.Exp, bias=maskT_sb[:, kt:kt + 1], scale=1.0)
            ex2 = expp.tile([P, 2, 512], f16, name="ex2", tag="ex2")
            nc.vector.tensor_mul(ex2, ex1, btw)
            for hi in range(2):
                h = 2 * pair + hi
                nc.tensor.matmul(
                    avs[qt][h],
                    lhsT=V_sb[:, kt, h, :],
                    rhs=ex2[:, hi, :],
                    start=(kt == 0), stop=(kt == KT - 1),
                )

        avcs = {}

        def emit_evac(qt):
            # evacuate the four AV psums to SBUF at the block boundary
            # (split across ACT and DVE to keep the convoy short); the
            # normalization chain is deferred into the next block
            avcs[qt] = []
            for h in range(4):
                avc = normp.tile([65, 512], f32, name="avc", tag="avc")
                if h < 2:
                    nc.scalar.copy(avc, avs[qt][h])
                else:
                    nc.vector.tensor_copy(avc, avs[qt][h])
                avcs[qt].append(avc)

        def emit_norm(qt):
            # Z broadcast via 2KB DRAM bounce + approx reciprocal
            for h in range(4):
                avc = avcs[qt][h]
                zscr = dramp.tile([1, 512], f32, name="zscr", tag="zscr")
                nc.sync.dma_start(zscr, avc[64:65, :])
                zb = normp.tile([64, 512], f32, name="zb", tag="zb")
                nc.sync.dma_start(
                    zb,
                    bass.AP(tensor=zscr.tensor, offset=zscr.offset,
                            ap=[[0, 64], [1, 512]]),
                )
                zr = normp.tile([64, 512], f32, name="zr", tag="zr")
                nc.vector.reciprocal_approx_fast(zr, zb)
                ot2 = normp.tile([64, 512], f32r, name="ot2", tag="ot2")
                nc.vector.tensor_mul(ot2, avc[0:64, :], zr)
                nc.gpsimd.dma_start(OTd[ts(h, 64), ts(qt, 512)], ot2)

        def emit_proj(qt):
            otl = otlp.tile([P, 2, 512], f32r, name="otl", tag="otl")
            nc.gpsimd.dma_start(
                otl, OTd[:, ts(qt, 512)].rearrange("(o p) l -> p o l", p=P))
            for jt in range(8):
                ps = psS.tile([P, 512], f32, name="ps_s", tag="ps_s")
                for cc in range(2):
                    nc.tensor.matmul(
                        ps,
                        lhsT=woutT_sb[:, cc, ts(jt, P)],
                        rhs=otl[:, cc, :],
                        start=(cc == 0), stop=(cc == 1),
                    )
                osb = outp.tile([P, 512], f32, name="osb", tag="osb")
                nc.vector.tensor_scalar_add(osb, ps, bout_sb[:, jt:jt + 1])
                nc.sync.dma_start(outT_d[ts(jt, P), ts(qt, 512)], osb)

        avs[0] = [psAV.tile([65, 512], f32, name=f"av0{hh}", tag="av")
                  for hh in range(4)]
        emit_kvproj(0)
        for kt3 in range(KT3):
            if kt3 + 1 < KT3:
                emit_kvproj(kt3 + 1)
            for kt in range(3 * kt3, min(3 * kt3 + 3, KT)):
                for pair in range(2):
                    emit_step(0, kt, pair)
        emit_evac(0)
        for qt in range(1, 4):
            avs[qt] = [psAV.tile([65, 512], f32, name=f"av{qt}{hh}", tag="av")
                       for hh in range(4)]
            for kt in range(KT):
                for pair in range(2):
                    emit_step(qt, kt, pair)
                if kt == 0:
                    emit_norm(qt - 1)
                if kt == 5:
                    emit_proj(qt - 1)
            emit_evac(qt)
        emit_norm(3)
        emit_proj(3)

    nc.compile()
    return nc


def _prep_core_inputs(c, Kp, x, key_padding_mask, attn_bias, W_in, b_in,
                      W_out, b_out):
    b, hg = c // HPC, c % HPC
    hs = slice(256 * hg, 256 * hg + 256)
    f32 = np.float32
    idx = np.where(~key_padding_mask[b])[0]
    nk = len(idx)
    wq, wk, wv = W_in[0:D][hs], W_in[D:2 * D][hs], W_in[2 * D:3 * D][hs]

    xk = np.zeros((Kp, D), dtype=f32)
    xk[:nk] = x[b][idx]
    # -ln(64) headroom shift: softmax is scale-invariant, and scaling all
    # exp weights by 1/64 keeps exp(s)*exp(bias) inside fp16 range.
    maskT = np.full(Kp, -10000.0, dtype=f32)
    maskT[:nk] = -np.log(64.0)
    biasT = np.zeros((HPC, Kp, L), dtype=np.float16)
    biasT[:, :nk, :] = np.exp(
        attn_bias[b, HPC * hg:HPC * hg + HPC][:, :, idx]
        .transpose(0, 2, 1)).astype(np.float16)

    return {
        "xT": np.ascontiguousarray(x[b].T, dtype=f32),
        "xkT": np.ascontiguousarray(xk.T, dtype=f32),
        "wqkT": np.ascontiguousarray(np.concatenate([wq, wk], 0).T, dtype=f32),
        "wvT": np.ascontiguousarray(wv.T, dtype=f32),
        "bqk": np.concatenate([b_in[0:D][hs] / 8.0, b_in[D:2 * D][hs]]).astype(f32),
        "maskT": maskT,
        "biasT": biasT,
        "woutT": np.ascontiguousarray(W_out[:, hs].T, dtype=f32),
        "bout4": (b_out / float(HPC) + W_out[:, hs] @ b_in[2 * D:3 * D][hs]).astype(f32),
    }


def kernel(x, key_padding_mask, attn_bias, W_in, b_in, W_out, b_out):
    global _compiled, LAST_RESULT
    from concourse.bass_utils import run_bass_kernel_spmd

    nk_max = int((~key_padding_mask).sum(axis=1).max())
    Kp = max(384, -(-nk_max // 384) * 384)

    if _compiled is None or _compiled[0] != Kp:
        _compiled = (Kp, _build(Kp))

    in_maps = [
        _prep_core_inputs(c, Kp, x, key_padding_mask, attn_bias, W_in, b_in,
                          W_out, b_out)
        for c in range(NCORES)
    ]
    res = run_bass_kernel_spmd(
        _compiled[1], in_maps, core_ids=list(range(NCORES)),
        trace_cores=(list(range(NCORES))
                     if os.environ.get("BASS_TRACE") == "1" else None),
    )
    LAST_RESULT = res

    out = np.empty((B, L, D), dtype=np.float32)
    for b in range(B):
        acc = res.results[b * HPC]["outT"].astype(np.float64)
        for g in range(1, HPC):
            acc = acc + res.results[b * HPC + g]["outT"]
        out[b] = acc.T.astype(np.float32)
    return out
